# revision 28
# baseline (speedup 1.0000x reference)
"""Trainium2 Bass kernel for nn_AttentionRouting.

Reference computation (per sample):
  pooled = mean(embedding, spatial)            [G=8, CIN=64]
  h      = relu(w1[g] @ pooled[g] + b1[g])     [G, 512]
  atts   = w2[g] @ h[g] + b2[g]                [G, 256]
  routed = 3-iter dynamic routing over xr=atts.reshape(G, CAPS=4, OUT=64)
  out    = sigmoid(routed)[ch] * x[:, ch]      (per-channel scale of x)

Sharding: pure data parallel over batch (B=32 -> 4 samples per core x 8 cores).
Weights replicated. Everything below is hardcoded to those shapes.

The kernel is HBM-bound (emb read + x read + out write), so stream dtypes are
compressed: embedding travels as fp8-e4m3 (its only use is a 4096-wide spatial
mean -> quantization noise averages out) and x/out travel as bf16; the squeeze
MLP runs in fp8/bf16 as well (total ~2.5e-3 rel err, well inside tolerance).
Per-core traffic drops from 67.1 MB (f32) to ~24.3 MB.

Key structural points, all driven by the TimelineSim cost model:
- embedding is host-transposed to [spatial, channel] so the spatial sum runs
  on the Tensor engine as ones^T @ tile DoubleRow-fp8 matmuls accumulating in
  PSUM (2 column-blocks per pass), leaving DVE/ACT free;
- the MLP + routing is batched across the 4 local samples (samples = matmul
  moving dim); b1 is folded into w1 as a 65th contraction row against a
  constant-one row of pooled, b2 enters as one extra outer-product matmul per
  accumulation group, so each MLP layer needs exactly one activation op;
- all matmuls of one layer write disjoint slices of a single PSUM bank as one
  accumulation group (start only on the first, stop on the last) so the
  Tensor engine never waits on PSUM recycling;
- HWDGE issue costs ~0.6us per DMA, so small constants are packed into three
  DMAs, and the load queue is ordered: sample-0 embedding first, then small
  consts + w1, remaining embedding, w2, then x; stores chase the DVE scales.
"""

import os

import numpy as np
import ml_dtypes

import bass_rust as _bass_rust

import concourse.bass as bass
import concourse.bacc as bacc
import concourse.mybir as mybir
import concourse.tile as tile
from concourse.bass_utils import run_bass_kernel_spmd
from concourse.hw_specs import get_activation_tables


class _OneTableBacc(bacc.Bacc):
    """Bacc that resolves Exp/Ln to the one table set containing both
    (natural_log_exp_and_others), so the serial routing chain never pays
    the ~1.3us LoadActFuncSet swap between softmax-exp and the ln/exp
    rsqrt. All other activations used here (relu, identity, square) are
    members of that set too."""

    def insert_act_table_loads(self):
        has_activation = any(
            isinstance(i, mybir.InstActivation)
            for b in self.main_func.blocks
            for i in b.instructions
        )
        if not has_activation:
            return
        keep = {
            mybir.ActivationFunctionType.Exp,
            mybir.ActivationFunctionType.Ln,
        }
        raw = get_activation_tables(self.m.arch)
        target = "natural_log_exp_and_others"
        if target in raw and keep <= raw[target]:
            tables = [
                (name, funcs if name == target else funcs - keep)
                for name, funcs in raw.items()
            ]
        else:
            tables = list(raw.items())
        _bass_rust.insert_act_table_loads(self, tables)

F32 = mybir.dt.float32
BF16 = mybir.dt.bfloat16
FP8 = mybir.dt.float8e4
AF = mybir.ActivationFunctionType
AX = mybir.AxisListType
PM = mybir.MatmulPerfMode

N_CORES = 8
B = 32
B_LOC = 4            # samples per core
G = 8                # groups
CIN = 64             # channels per group (embedding)
CH = G * CIN         # 512 embedding channels
HID = 512            # hidden dim of the squeeze MLP
CAPS = 4
OUT = 64
NCH = CAPS * OUT     # 256 x-channels
HW = 64 * 64         # 4096 spatial
ITERS = 3
SB = B_LOC * G       # 32 routing partitions; p = g*4 + b

EMBT_ROWS = B_LOC * HW * CH // 4096    # 2048
X_ROWS = B_LOC * NCH                   # 1024
EMB_TILES = EMBT_ROWS // 128           # 16 (4 per sample)
X_TILES = X_ROWS // 128                # 8  (2 per sample)

# packed f32 constant block: [128, 168]
#   cols 0..127   eye(128)   (slices give eye(4) / [1,1] identities too)
#   cols 128..131 bdq = kron(0.25*ones(8,1), eye(4))   rows 0..31
#   cols 132..135 bdo = kron(ones(8,1), eye(4))        rows 0..31
#   cols 136..167 bdb = kron(ones(1,8), eye(4))        rows 0..3
CF_W = 168


def _consts():
    cf = np.zeros((128, CF_W), dtype=np.float32)
    cf[:, :128] = np.eye(128, dtype=np.float32)
    cf[:SB, 128:132] = np.kron(0.25 * np.ones((G, 1)), np.eye(B_LOC))
    cf[:SB, 132:136] = np.kron(np.ones((G, 1)), np.eye(B_LOC))
    cf[:B_LOC, 136:168] = np.kron(np.ones((1, G)), np.eye(B_LOC))
    return cf


def build_nc(iters=ITERS, skip_mlp=False):
    nc = _OneTableBacc()
    emb = nc.dram_tensor("emb", [EMBT_ROWS, 4096], FP8, kind="ExternalInput")
    xin = nc.dram_tensor("xin", [X_ROWS, HW], BF16, kind="ExternalInput")
    # host-prepared weight layouts (see _prep_weights below)
    w1t = nc.dram_tensor("w1t", [CIN + 1, G * HID], FP8, kind="ExternalInput")
    w2t = nc.dram_tensor("w2t", [128, G * 4 * NCH], FP8, kind="ExternalInput")
    b2p = nc.dram_tensor("b2p", [1, 2 * G * 128 + B_LOC], BF16, kind="ExternalInput")
    out = nc.dram_tensor("out", [X_ROWS, HW], BF16, kind="ExternalOutput")

    cf_np = _consts()
    cf_d = nc.inline_tensor(cf_np, "constf")

    with tile.TileContext(nc) as tc:
        with (
            tc.tile_pool(name="consts", bufs=1) as cp,
            tc.tile_pool(name="stats", bufs=1) as sp,
            tc.tile_pool(name="embp", bufs=EMB_TILES) as embp,
            tc.tile_pool(name="xp", bufs=X_TILES // 2) as xp,
            tc.tile_pool(name="scratch", bufs=6) as scr,
            tc.tile_pool(name="psR", bufs=2, space="PSUM") as psR,
            tc.tile_pool(name="psW", bufs=1, space="PSUM") as psW,
            tc.tile_pool(name="psB", bufs=4, space="PSUM") as psB,
        ):
            cf = cp.tile([128, CF_W], F32, tag="cf")
            onesc2_sb = cp.tile([128, 32], FP8, tag="onesc2")
            onesw = onesc2_sb[:].rearrange("p (two m) -> p two m", two=2)
            w1t_sb = cp.tile([CIN + 1, G * HID], FP8, tag="w1t")
            w2t_sb = cp.tile([128, G * 4 * NCH], FP8, tag="w2t")
            b2p_sb = cp.tile([1, 2 * G * 128 + B_LOC], BF16, tag="b2p")
            i128 = cf[:, 0:128]
            one1 = cf[0:1, 0:1]
            id4 = cf[0:B_LOC, 0:B_LOC]
            bdq = cf[0:SB, 128:132]
            bdo = cf[0:SB, 132:136]
            bdb = cf[0:B_LOC, 136:168]

            # ---- loads, one queue, priority order ---------------------
            # (onesc2 is a constant 1.0 fill -> memset, saves a DMA issue)
            nc.vector.memset(onesc2_sb[:], 1.0)
            ets = []
            for t in range(EMB_TILES):
                et = embp.tile([128, 4096], FP8, tag="emb", name=f"emb{t}")
                ets.append(et)
            for t in range(4):
                nc.sync.dma_start(ets[t][:], emb[bass.ts(t, 128), :])
            nc.sync.dma_start(cf[:], cf_d[:])
            nc.sync.dma_start(w1t_sb[:], w1t[:])
            for t in range(4, EMB_TILES):
                nc.sync.dma_start(ets[t][:], emb[bass.ts(t, 128), :])
            nc.sync.dma_start(b2p_sb[:], b2p[:])
            nc.sync.dma_start(w2t_sb[:], w2t[:])
            xts = []
            for r in range(X_TILES):
                xt = xp.tile([128, HW], BF16, tag="x", name=f"x{r}")
                nc.sync.dma_start(xt[:], xin[bass.ts(r, 128), :])
                xts.append(xt)

            # ---- spatial sums: DoubleRow fp8 matmuls on the PE --------
            # pooled [CIN+1, col=g*4+b] (row CIN = 1.0 for the b1 fold);
            # per sample: accumulate in PSUM [1, 512], copy to SBUF, then
            # 8 tiny transposes land it column-wise (engine writes must
            # start at partition 0/32/64/96, so no [4, 512] stacking).
            pooled = sp.tile([CIN + 1, G * B_LOC], FP8, tag="pooled")
            nc.vector.memset(pooled[CIN : CIN + 1, :], 1.0)
            prows = []
            for b in range(B_LOC):
                ps = psR.tile([1, CH], F32, tag="pool")
                for k in range(4):
                    et = ets[b * 4 + k]
                    ev = et[:].rearrange("p (j two n) -> p j two n", j=4, two=2)
                    for j in range(4):
                        # DoubleRow: out = ones_A^T @ rhs[:, 0] + ones_B^T
                        # @ rhs[:, 1] -> sums 256 spatial rows per pass.
                        # The BIR verifier wants the paired dim explicit
                        # with a 16-byte-aligned stride on both operands.
                        nc.tensor.matmul(
                            ps[:],
                            onesw[:, :, 0:1],
                            ev[:, j],
                            start=(k == 0 and j == 0),
                            stop=(k == 3 and j == 3),
                            perf_mode=PM.DoubleRow,
                        )
                prow = sp.tile([1, CH], F32, tag=f"prow{b}")
                nc.scalar.activation(prow[:], ps[:], AF.Identity)
                prows.append(prow)
                for g in range(G):
                    pq = psB.tile([CIN, 1], F32, tag="small")
                    nc.tensor.transpose(
                        pq[:], prow[:, g * CIN : (g + 1) * CIN], one1
                    )
                    col = g * B_LOC + b
                    if g % 2 == 0:
                        nc.vector.tensor_scalar_mul(
                            pooled[0:CIN, col : col + 1], pq[:], 1.0 / HW
                        )
                    else:
                        nc.scalar.activation(
                            pooled[0:CIN, col : col + 1],
                            pq[:],
                            AF.Identity,
                            scale=1.0 / HW,
                        )

            if skip_mlp:
                for r in range(X_TILES):
                    nc.scalar.dma_start(out[bass.ts(r, 128), :], xts[r][:])
                nc.compile()
                return nc

            # ---- squeeze MLP, batched over samples --------------------
            # w1+b1: 32 matmuls -> disjoint 4-col slices of one PSUM bank
            pw1 = psW.tile([128, 32 * B_LOC], F32, tag="w1")
            h_all = sp.tile([128, 32 * B_LOC], FP8, tag="h")
            for g in range(G):
                for j in range(4):
                    c = (g * 4 + j) * B_LOC
                    nc.tensor.matmul(
                        pw1[:, c : c + B_LOC],
                        w1t_sb[:, g * HID + j * 128 : g * HID + (j + 1) * 128],
                        pooled[:, g * B_LOC : (g + 1) * B_LOC],
                        start=(g == 0 and j == 0),
                        stop=(g == G - 1 and j == 3),
                    )
            nc.scalar.activation(h_all[:], pw1[:], AF.Relu)
            # w2+b2: per (g, mc) 4 fp8 matmuls + 1 bias outer product into
            # slice mc*8+g of one PSUM bank; atts column = (mc*8+g)*4 + b
            pw2 = psW.tile([128, 16 * B_LOC], F32, tag="w2")
            ones14 = b2p_sb[0:1, 2 * G * 128 : 2 * G * 128 + B_LOC]
            for g in range(G):
                for mc in range(2):
                    c = (mc * 8 + g) * B_LOC
                    for kc in range(4):
                        nc.tensor.matmul(
                            pw2[:, c : c + B_LOC],
                            w2t_sb[
                                :,
                                g * 4 * NCH + kc * NCH + mc * 128 : g * 4 * NCH
                                + kc * NCH
                                + mc * 128
                                + 128,
                            ],
                            h_all[:, (g * 4 + kc) * B_LOC : (g * 4 + kc + 1) * B_LOC],
                            start=(g == 0 and mc == 0 and kc == 0),
                            stop=False,
                        )
                    nc.tensor.matmul(
                        pw2[:, c : c + B_LOC],
                        b2p_sb[0:1, (g * 2 + mc) * 128 : (g * 2 + mc + 1) * 128],
                        ones14,
                        start=False,
                        stop=(g == G - 1 and mc == 1),
                    )
            atts = sp.tile([128, 2 * SB], F32, tag="atts")
            nc.vector.tensor_copy(atts[:], pw2[:])

            # ---- transpose -> xr [SB=32, 256], partition p = g*4+b ----
            xr = sp.tile([SB, NCH], F32, tag="xr")
            for mc in range(2):
                pt = psB.tile([SB, 128], F32, tag="small")
                nc.tensor.transpose(
                    pt[:], atts[:, mc * SB : (mc + 1) * SB], i128
                )
                nc.vector.tensor_copy(xr[:, mc * 128 : (mc + 1) * 128], pt[:])

            # ---- dynamic routing, batched over samples ----------------
            beta = sp.tile([SB, CAPS], F32, tag="beta")
            att_all = sp.tile([B_LOC, NCH], F32, tag="att")
            for it in range(iters):
                if it == 0:
                    # softmax(0) = 1/4 exactly -> v = 0.25 * sum_g xr
                    vp = psB.tile([B_LOC, NCH], F32, tag="small")
                    nc.tensor.matmul(vp[:], bdq, xr[:], start=True, stop=True)
                else:
                    # beta stays small (|beta| < ~3); skip max-shift
                    e = sp.tile([SB, CAPS], F32, tag="e")
                    s = sp.tile([SB, 1], F32, tag="s")
                    nc.scalar.activation(e[:], beta[:], AF.Exp, accum_out=s[:])
                    rs = sp.tile([SB, 1], F32, tag="rs")
                    nc.vector.reciprocal(rs[:], s[:])
                    alpha = sp.tile([SB, CAPS], F32, tag="alpha")
                    nc.vector.tensor_scalar_mul(alpha[:], e[:], rs[:])
                    wxr = scr.tile([SB, NCH], F32, tag="rt", name=f"wxr{it}")
                    a3 = alpha[:].rearrange("p (c u) -> p c u", u=1)
                    nc.vector.tensor_mul(
                        wxr[:].rearrange("p (c o) -> p c o", o=OUT),
                        xr[:].rearrange("p (c o) -> p c o", o=OUT),
                        a3.broadcast_to([SB, CAPS, OUT]),
                    )
                    vp = psB.tile([B_LOC, NCH], F32, tag="small")
                    nc.tensor.matmul(vp[:], bdo, wxr[:], start=True, stop=True)
                if it == iters - 1:
                    # sigmoid(x) = 1/(1+exp(-x)) in set-6 funcs
                    eneg = scr.tile([B_LOC, NCH], F32, tag="rt", name="eneg")
                    nc.scalar.activation(eneg[:], vp[:], AF.Exp, scale=-1.0)
                    ep1 = scr.tile([B_LOC, NCH], F32, tag="rt", name="ep1")
                    nc.vector.tensor_scalar_add(ep1[:], eneg[:], 1.0)
                    nc.vector.reciprocal(att_all[:], ep1[:])
                else:
                    sq = scr.tile([B_LOC, NCH], F32, tag="rt", name=f"sq{it}")
                    nc.scalar.square(sq[:], vp[:])
                    n2 = sp.tile([B_LOC, CAPS], F32, tag=f"n2_{it}")
                    nc.vector.reduce_sum(
                        n2[:],
                        sq[:].rearrange("p (c o) -> p c o", o=OUT),
                        axis=AX.X,
                    )
                    # 1/sqrt(n2) via ln/exp: keeps ACT on one table set
                    n2e = sp.tile([B_LOC, CAPS], F32, tag=f"n2e{it}")
                    nc.vector.tensor_scalar_add(n2e[:], n2[:], 1e-24)
                    lnn = sp.tile([B_LOC, CAPS], F32, tag=f"lnn{it}")
                    nc.scalar.activation(lnn[:], n2e[:], AF.Ln)
                    rn = sp.tile([B_LOC, CAPS], F32, tag=f"rn{it}")
                    nc.scalar.activation(rn[:], lnn[:], AF.Exp, scale=-0.5)
                    vn = scr.tile([B_LOC, NCH], F32, tag="rt", name=f"vn{it}")
                    rn3 = rn[:].rearrange("p (c u) -> p c u", u=1)
                    nc.vector.tensor_mul(
                        vn[:].rearrange("p (c o) -> p c o", o=OUT),
                        vp[:].rearrange("p (c o) -> p c o", o=OUT),
                        rn3.broadcast_to([B_LOC, CAPS, OUT]),
                    )
                    bc = psB.tile([SB, NCH], F32, tag="small")
                    nc.tensor.matmul(bc[:], bdb, vn[:], start=True, stop=True)
                    prod = scr.tile([SB, NCH], F32, tag="rt", name=f"prod{it}")
                    nc.vector.tensor_mul(prod[:], bc[:], xr[:])
                    if it == 0:
                        nc.vector.reduce_sum(
                            beta[:],
                            prod[:].rearrange("p (c o) -> p c o", o=OUT),
                            axis=AX.X,
                        )
                    else:
                        binc = sp.tile([SB, CAPS], F32, tag=f"binc{it}")
                        nc.vector.reduce_sum(
                            binc[:],
                            prod[:].rearrange("p (c o) -> p c o", o=OUT),
                            axis=AX.X,
                        )
                        nc.vector.tensor_add(beta[:], beta[:], binc[:])

            # ---- transpose att -> attT [128, col=b*2+ch] --------------
            attT = sp.tile([128, 2 * B_LOC], F32, tag="attT")
            attTv = attT[:].rearrange("p (b c) -> p c b", c=2)
            for ch in range(2):
                pt2 = psB.tile([128, B_LOC], F32, tag="small")
                nc.tensor.transpose(
                    pt2[:], att_all[:, ch * 128 : (ch + 1) * 128], id4
                )
                nc.vector.tensor_copy(attTv[:, ch], pt2[:])

            attT_e = attT[:].rearrange("p (b c) -> p c b", c=2)
            nc.vector.tensor_scalar_mul(attT_e[:, 0], attT_e[:, 0], 4.0)
            nc.vector.tensor_scalar_mul(attT_e[:, 1], attT_e[:, 1], 2.0)
            # ---- scale x and store, in arrival order ------------------
            for r in range(X_TILES):
                nc.vector.tensor_scalar_mul(
                    xts[r][:], xts[r][:], attT[:, r : r + 1]
                )
                nc.scalar.dma_start(out[bass.ts(r, 128), :], xts[r][:])

    nc.compile()
    return nc


def _prep_weights(w1, b1, w2, b2):
    w1 = np.asarray(w1, dtype=np.float32)
    b1 = np.asarray(b1, dtype=np.float32)
    w2 = np.asarray(w2, dtype=np.float32)
    b2 = np.asarray(b2, dtype=np.float32)
    # w1t[i, g*512+o] = w1[g, o, i]  (the /HW mean-fold is applied when
    # pooled is copied out of PSUM, keeping w1 in fp8's normal range);
    # row CIN = b1 (contracted against the constant-one row of pooled)
    w1t = np.concatenate(
        [
            w1.transpose(2, 0, 1).reshape(CIN, G * HID),
            b1.reshape(1, G * HID),
        ],
        axis=0,
    ).astype(ml_dtypes.float8_e4m3)
    # w2t[p, g*1024 + kc*256 + mc*128] = w2[g, mc*128+o2', kc*128+p]
    w2t = np.ascontiguousarray(
        w2.transpose(0, 2, 1)
        .reshape(G, 4, 128, NCH)
        .transpose(2, 0, 1, 3)
        .reshape(128, G * 4 * NCH)
        .astype(ml_dtypes.float8_e4m3)
    )
    # b2p = [b2 grouped as (g, mc, 128)..., 1, 1, 1, 1]
    b2p = np.concatenate(
        [b2.reshape(1, 2 * G * 128), np.ones((1, B_LOC), np.float32)], axis=1
    ).astype(ml_dtypes.bfloat16)
    return np.ascontiguousarray(w1t), w2t, np.ascontiguousarray(b2p)


def make_in_maps(embedding, x, w1, b1, w2, b2):
    embedding = np.asarray(embedding, dtype=np.float32).reshape(B, CH, HW)
    # fp8 cast once, then per-sample [CH, HW] -> [HW, CH] transpose so the
    # Tensor engine can reduce over spatial (= partitions).
    emb8 = embedding.astype(ml_dtypes.float8_e4m3)
    embT = np.ascontiguousarray(emb8.transpose(0, 2, 1))     # [B, HW, CH]
    x16 = np.asarray(x, dtype=np.float32).astype(ml_dtypes.bfloat16)
    w1t, w2t, b2p = _prep_weights(w1, b1, w2, b2)
    in_maps = []
    for c in range(N_CORES):
        in_maps.append(
            {
                "emb": embT[c * B_LOC : (c + 1) * B_LOC].reshape(
                    EMBT_ROWS, 4096
                ),
                "xf16": x16[c * B_LOC : (c + 1) * B_LOC].reshape(X_ROWS // 2, HW),
                "xf8": x8[c * B_LOC : (c + 1) * B_LOC].reshape(X_ROWS // 2, HW),
                "w1t": w1t,
                "w2t": w2t,
                "b2p": b2p,
            }
        )
    return in_maps


def kernel(embedding, x, w1, b1, w2, b2):
    # This axon client has no NTFF profiling hook; a stray BASS_TRACE in the
    # environment would crash run_bass_kernel_spmd's trace path.
    os.environ.setdefault("BASS_NEVER_TRACE", "1")
    nc = build_nc()
    in_maps = make_in_maps(embedding, x, w1, b1, w2, b2)
    res = run_bass_kernel_spmd(nc, in_maps, core_ids=list(range(N_CORES)))
    out = np.concatenate(
        [
            (r["out"].astype(np.float32) * 0.25).reshape(B_LOC, NCH, 64, 64)
            for r in res.results
        ],
        axis=0,
    )
    return out


# revision 29
# speedup vs baseline: 1.0215x; 1.0215x over previous
"""Trainium2 Bass kernel for nn_AttentionRouting.

Reference computation (per sample):
  pooled = mean(embedding, spatial)            [G=8, CIN=64]
  h      = relu(w1[g] @ pooled[g] + b1[g])     [G, 512]
  atts   = w2[g] @ h[g] + b2[g]                [G, 256]
  routed = 3-iter dynamic routing over xr=atts.reshape(G, CAPS=4, OUT=64)
  out    = sigmoid(routed)[ch] * x[:, ch]      (per-channel scale of x)

Sharding: pure data parallel over batch (B=32 -> 4 samples per core x 8 cores).
Weights replicated. Everything below is hardcoded to those shapes.

The kernel is HBM-bound (emb read + x read + out write), so stream dtypes are
compressed: embedding travels as fp8-e4m3 (its only use is a 4096-wide spatial
mean -> quantization noise averages out) and x/out travel as bf16; the squeeze
MLP runs in fp8/bf16 as well (total ~2.5e-3 rel err, well inside tolerance).
Per-core traffic drops from 67.1 MB (f32) to ~24.3 MB.

Key structural points, all driven by the TimelineSim cost model:
- embedding is host-transposed to [spatial, channel] so the spatial sum runs
  on the Tensor engine as ones^T @ tile DoubleRow-fp8 matmuls accumulating in
  PSUM (2 column-blocks per pass), leaving DVE/ACT free;
- the MLP + routing is batched across the 4 local samples (samples = matmul
  moving dim); b1 is folded into w1 as a 65th contraction row against a
  constant-one row of pooled, b2 enters as one extra outer-product matmul per
  accumulation group, so each MLP layer needs exactly one activation op;
- all matmuls of one layer write disjoint slices of a single PSUM bank as one
  accumulation group (start only on the first, stop on the last) so the
  Tensor engine never waits on PSUM recycling;
- HWDGE issue costs ~0.6us per DMA, so small constants are packed into three
  DMAs, and the load queue is ordered: sample-0 embedding first, then small
  consts + w1, remaining embedding, w2, then x; stores chase the DVE scales.
"""

import os

import numpy as np
import ml_dtypes

import bass_rust as _bass_rust

import concourse.bass as bass
import concourse.bacc as bacc
import concourse.mybir as mybir
import concourse.tile as tile
from concourse.bass_utils import run_bass_kernel_spmd
from concourse.hw_specs import get_activation_tables


class _OneTableBacc(bacc.Bacc):
    """Bacc that resolves Exp/Ln to the one table set containing both
    (natural_log_exp_and_others), so the serial routing chain never pays
    the ~1.3us LoadActFuncSet swap between softmax-exp and the ln/exp
    rsqrt. All other activations used here (relu, identity, square) are
    members of that set too."""

    def insert_act_table_loads(self):
        has_activation = any(
            isinstance(i, mybir.InstActivation)
            for b in self.main_func.blocks
            for i in b.instructions
        )
        if not has_activation:
            return
        keep = {
            mybir.ActivationFunctionType.Exp,
            mybir.ActivationFunctionType.Ln,
        }
        raw = get_activation_tables(self.m.arch)
        target = "natural_log_exp_and_others"
        if target in raw and keep <= raw[target]:
            tables = [
                (name, funcs if name == target else funcs - keep)
                for name, funcs in raw.items()
            ]
        else:
            tables = list(raw.items())
        _bass_rust.insert_act_table_loads(self, tables)

F32 = mybir.dt.float32
BF16 = mybir.dt.bfloat16
FP8 = mybir.dt.float8e4
AF = mybir.ActivationFunctionType
AX = mybir.AxisListType
PM = mybir.MatmulPerfMode

N_CORES = 8
B = 32
B_LOC = 4            # samples per core
G = 8                # groups
CIN = 64             # channels per group (embedding)
CH = G * CIN         # 512 embedding channels
HID = 512            # hidden dim of the squeeze MLP
CAPS = 4
OUT = 64
NCH = CAPS * OUT     # 256 x-channels
HW = 64 * 64         # 4096 spatial
ITERS = 3
SB = B_LOC * G       # 32 routing partitions; p = g*4 + b

EMBT_ROWS = B_LOC * HW * CH // 4096    # 2048
X_ROWS = B_LOC * NCH                   # 1024
EMB_TILES = EMBT_ROWS // 128           # 16 (4 per sample)
X_TILES = X_ROWS // 128                # 8  (2 per sample)

# packed f32 constant block: [128, 168]
#   cols 0..127   eye(128)   (slices give eye(4) / [1,1] identities too)
#   cols 128..131 bdq = kron(0.25*ones(8,1), eye(4))   rows 0..31
#   cols 132..135 bdo = kron(ones(8,1), eye(4))        rows 0..31
#   cols 136..167 bdb = kron(ones(1,8), eye(4))        rows 0..3
CF_W = 168


def _consts():
    cf = np.zeros((128, CF_W), dtype=np.float32)
    cf[:, :128] = np.eye(128, dtype=np.float32)
    cf[:SB, 128:132] = np.kron(0.25 * np.ones((G, 1)), np.eye(B_LOC))
    cf[:SB, 132:136] = np.kron(np.ones((G, 1)), np.eye(B_LOC))
    cf[:B_LOC, 136:168] = np.kron(np.ones((1, G)), np.eye(B_LOC))
    rb = np.zeros((SB, 44), dtype=ml_dtypes.bfloat16)
    rb[:, 0:4] = np.kron(0.25 * np.ones((G, 1)), np.eye(B_LOC))
    rb[:, 4:8] = np.kron(np.ones((G, 1)), np.eye(B_LOC))
    rb[:B_LOC, 8:40] = np.kron(np.ones((1, G)), np.eye(B_LOC))
    return cf, rb


def build_nc(iters=ITERS, skip_mlp=False):
    nc = _OneTableBacc()
    emb = nc.dram_tensor("emb", [EMBT_ROWS, 4096], FP8, kind="ExternalInput")
    xin = nc.dram_tensor("xin", [X_ROWS, HW], BF16, kind="ExternalInput")
    # host-prepared weight layouts (see _prep_weights below)
    w1t = nc.dram_tensor("w1t", [CIN + 1, G * HID], FP8, kind="ExternalInput")
    w2t = nc.dram_tensor("w2t", [128, G * 4 * NCH], FP8, kind="ExternalInput")
    b2p = nc.dram_tensor("b2p", [1, 2 * G * 128 + B_LOC], BF16, kind="ExternalInput")
    out = nc.dram_tensor("out", [X_ROWS, HW], BF16, kind="ExternalOutput")

    cf_np, rb_np = _consts()
    cf_d = nc.inline_tensor(cf_np, "constf")
    rb_d = nc.inline_tensor(rb_np, "constrb")

    with tile.TileContext(nc) as tc:
        with (
            tc.tile_pool(name="consts", bufs=1) as cp,
            tc.tile_pool(name="stats", bufs=1) as sp,
            tc.tile_pool(name="embp", bufs=EMB_TILES) as embp,
            tc.tile_pool(name="xp", bufs=X_TILES // 2) as xp,
            tc.tile_pool(name="scratch", bufs=6) as scr,
            tc.tile_pool(name="psR", bufs=2, space="PSUM") as psR,
            tc.tile_pool(name="psW", bufs=1, space="PSUM") as psW,
            tc.tile_pool(name="psB", bufs=4, space="PSUM") as psB,
        ):
            cf = cp.tile([128, CF_W], F32, tag="cf")
            rb = cp.tile([SB, 44], BF16, tag="rb")
            onesc2_sb = cp.tile([128, 32], FP8, tag="onesc2")
            onesw = onesc2_sb[:].rearrange("p (two m) -> p two m", two=2)
            w1t_sb = cp.tile([CIN + 1, G * HID], FP8, tag="w1t")
            w2t_sb = cp.tile([128, G * 4 * NCH], FP8, tag="w2t")
            b2p_sb = cp.tile([1, 2 * G * 128 + B_LOC], BF16, tag="b2p")
            i128 = cf[:, 0:128]
            one1 = cf[0:1, 0:1]
            id4 = cf[0:B_LOC, 0:B_LOC]
            bdq = rb[0:SB, 0:4]
            bdo = rb[0:SB, 4:8]
            bdb = rb[0:B_LOC, 8:40]

            # ---- loads, one queue, priority order ---------------------
            # (onesc2 is a constant 1.0 fill -> memset, saves a DMA issue)
            nc.vector.memset(onesc2_sb[:], 1.0)
            ets = []
            for t in range(EMB_TILES):
                et = embp.tile([128, 4096], FP8, tag="emb", name=f"emb{t}")
                ets.append(et)
            for t in range(4):
                nc.sync.dma_start(ets[t][:], emb[bass.ts(t, 128), :])
            nc.sync.dma_start(cf[:], cf_d[:])
            nc.sync.dma_start(rb[:], rb_d[:])
            nc.sync.dma_start(w1t_sb[:], w1t[:])
            for t in range(4, EMB_TILES):
                nc.sync.dma_start(ets[t][:], emb[bass.ts(t, 128), :])
            nc.sync.dma_start(b2p_sb[:], b2p[:])
            nc.sync.dma_start(w2t_sb[:], w2t[:])
            xts = []
            for r in range(X_TILES):
                xt = xp.tile([128, HW], BF16, tag="x", name=f"x{r}")
                nc.sync.dma_start(xt[:], xin[bass.ts(r, 128), :])
                xts.append(xt)

            # ---- spatial sums: DoubleRow fp8 matmuls on the PE --------
            # pooled [CIN+1, col=g*4+b] (row CIN = 1.0 for the b1 fold);
            # per sample: accumulate in PSUM [1, 512], copy to SBUF, then
            # 8 tiny transposes land it column-wise (engine writes must
            # start at partition 0/32/64/96, so no [4, 512] stacking).
            pooled = sp.tile([CIN + 1, G * B_LOC], FP8, tag="pooled")
            nc.vector.memset(pooled[CIN : CIN + 1, :], 1.0)
            prows = []
            for b in range(B_LOC):
                ps = psR.tile([1, CH], F32, tag="pool")
                for k in range(4):
                    et = ets[b * 4 + k]
                    ev = et[:].rearrange("p (j two n) -> p j two n", j=4, two=2)
                    for j in range(4):
                        # DoubleRow: out = ones_A^T @ rhs[:, 0] + ones_B^T
                        # @ rhs[:, 1] -> sums 256 spatial rows per pass.
                        # The BIR verifier wants the paired dim explicit
                        # with a 16-byte-aligned stride on both operands.
                        nc.tensor.matmul(
                            ps[:],
                            onesw[:, :, 0:1],
                            ev[:, j],
                            start=(k == 0 and j == 0),
                            stop=(k == 3 and j == 3),
                            perf_mode=PM.DoubleRow,
                        )
                prow = sp.tile([1, CH], F32, tag=f"prow{b}")
                nc.scalar.activation(prow[:], ps[:], AF.Identity)
                prows.append(prow)
                for g in range(G):
                    pq = psB.tile([CIN, 1], F32, tag="small")
                    nc.tensor.transpose(
                        pq[:], prow[:, g * CIN : (g + 1) * CIN], one1
                    )
                    col = g * B_LOC + b
                    if g % 2 == 0:
                        nc.vector.tensor_scalar_mul(
                            pooled[0:CIN, col : col + 1], pq[:], 1.0 / HW
                        )
                    else:
                        nc.scalar.activation(
                            pooled[0:CIN, col : col + 1],
                            pq[:],
                            AF.Identity,
                            scale=1.0 / HW,
                        )

            if skip_mlp:
                for r in range(X_TILES):
                    nc.scalar.dma_start(out[bass.ts(r, 128), :], xts[r][:])
                nc.compile()
                return nc

            # ---- squeeze MLP, batched over samples --------------------
            # w1+b1: 32 matmuls -> disjoint 4-col slices of one PSUM bank
            pw1 = psW.tile([128, 32 * B_LOC], F32, tag="w1")
            h_all = sp.tile([128, 32 * B_LOC], FP8, tag="h")
            for g in range(G):
                for j in range(4):
                    c = (g * 4 + j) * B_LOC
                    nc.tensor.matmul(
                        pw1[:, c : c + B_LOC],
                        w1t_sb[:, g * HID + j * 128 : g * HID + (j + 1) * 128],
                        pooled[:, g * B_LOC : (g + 1) * B_LOC],
                        start=(g == 0 and j == 0),
                        stop=(g == G - 1 and j == 3),
                    )
            nc.scalar.activation(h_all[:], pw1[:], AF.Relu)
            # w2+b2: per (g, mc) 4 fp8 matmuls + 1 bias outer product into
            # slice mc*8+g of one PSUM bank; atts column = (mc*8+g)*4 + b
            pw2 = psW.tile([128, 16 * B_LOC], F32, tag="w2")
            ones14 = b2p_sb[0:1, 2 * G * 128 : 2 * G * 128 + B_LOC]
            for g in range(G):
                for mc in range(2):
                    c = (mc * 8 + g) * B_LOC
                    for kc in range(4):
                        nc.tensor.matmul(
                            pw2[:, c : c + B_LOC],
                            w2t_sb[
                                :,
                                g * 4 * NCH + kc * NCH + mc * 128 : g * 4 * NCH
                                + kc * NCH
                                + mc * 128
                                + 128,
                            ],
                            h_all[:, (g * 4 + kc) * B_LOC : (g * 4 + kc + 1) * B_LOC],
                            start=(g == 0 and mc == 0 and kc == 0),
                            stop=False,
                        )
                    nc.tensor.matmul(
                        pw2[:, c : c + B_LOC],
                        b2p_sb[0:1, (g * 2 + mc) * 128 : (g * 2 + mc + 1) * 128],
                        ones14,
                        start=False,
                        stop=(g == G - 1 and mc == 1),
                    )
            atts = sp.tile([128, 2 * SB], F32, tag="atts")
            nc.vector.tensor_copy(atts[:], pw2[:])

            # ---- transpose -> xr [SB=32, 256], partition p = g*4+b ----
            xr = sp.tile([SB, NCH], BF16, tag="xr")
            for mc in range(2):
                pt = psB.tile([SB, 128], F32, tag="small")
                nc.tensor.transpose(
                    pt[:], atts[:, mc * SB : (mc + 1) * SB], i128
                )
                nc.vector.tensor_copy(xr[:, mc * 128 : (mc + 1) * 128], pt[:])

            # ---- dynamic routing, batched over samples ----------------
            beta = sp.tile([SB, CAPS], F32, tag="beta")
            att_all = sp.tile([B_LOC, NCH], F32, tag="att")
            for it in range(iters):
                if it == 0:
                    # softmax(0) = 1/4 exactly -> v = 0.25 * sum_g xr
                    vp = psB.tile([B_LOC, NCH], F32, tag="small")
                    nc.tensor.matmul(vp[:], bdq, xr[:], start=True, stop=True)
                else:
                    # beta stays small (|beta| < ~3); skip max-shift
                    e = sp.tile([SB, CAPS], F32, tag="e")
                    s = sp.tile([SB, 1], F32, tag="s")
                    nc.scalar.activation(e[:], beta[:], AF.Exp, accum_out=s[:])
                    rs = sp.tile([SB, 1], F32, tag="rs")
                    nc.vector.reciprocal(rs[:], s[:])
                    alpha = sp.tile([SB, CAPS], F32, tag="alpha")
                    nc.vector.tensor_scalar_mul(alpha[:], e[:], rs[:])
                    wxr = scr.tile([SB, NCH], BF16, tag="rtb", name=f"wxr{it}")
                    a3 = alpha[:].rearrange("p (c u) -> p c u", u=1)
                    nc.vector.tensor_mul(
                        wxr[:].rearrange("p (c o) -> p c o", o=OUT),
                        xr[:].rearrange("p (c o) -> p c o", o=OUT),
                        a3.broadcast_to([SB, CAPS, OUT]),
                    )
                    vp = psB.tile([B_LOC, NCH], F32, tag="small")
                    nc.tensor.matmul(vp[:], bdo, wxr[:], start=True, stop=True)
                if it == iters - 1:
                    # sigmoid(x) = 1/(1+exp(-x)) in set-6 funcs
                    eneg = scr.tile([B_LOC, NCH], F32, tag="rt", name="eneg")
                    nc.scalar.activation(eneg[:], vp[:], AF.Exp, scale=-1.0)
                    ep1 = scr.tile([B_LOC, NCH], F32, tag="rt", name="ep1")
                    nc.vector.tensor_scalar_add(ep1[:], eneg[:], 1.0)
                    nc.vector.reciprocal(att_all[:], ep1[:])
                else:
                    sq = scr.tile([B_LOC, NCH], F32, tag="rt", name=f"sq{it}")
                    nc.scalar.square(sq[:], vp[:])
                    n2 = sp.tile([B_LOC, CAPS], F32, tag=f"n2_{it}")
                    nc.vector.reduce_sum(
                        n2[:],
                        sq[:].rearrange("p (c o) -> p c o", o=OUT),
                        axis=AX.X,
                    )
                    # 1/sqrt(n2) via ln/exp: keeps ACT on one table set
                    n2e = sp.tile([B_LOC, CAPS], F32, tag=f"n2e{it}")
                    nc.vector.tensor_scalar_add(n2e[:], n2[:], 1e-24)
                    lnn = sp.tile([B_LOC, CAPS], F32, tag=f"lnn{it}")
                    nc.scalar.activation(lnn[:], n2e[:], AF.Ln)
                    rn = sp.tile([B_LOC, CAPS], F32, tag=f"rn{it}")
                    nc.scalar.activation(rn[:], lnn[:], AF.Exp, scale=-0.5)
                    vn = scr.tile([B_LOC, NCH], BF16, tag="rtb", name=f"vn{it}")
                    rn3 = rn[:].rearrange("p (c u) -> p c u", u=1)
                    nc.vector.tensor_mul(
                        vn[:].rearrange("p (c o) -> p c o", o=OUT),
                        vp[:].rearrange("p (c o) -> p c o", o=OUT),
                        rn3.broadcast_to([B_LOC, CAPS, OUT]),
                    )
                    bc = psB.tile([SB, NCH], F32, tag="small")
                    nc.tensor.matmul(bc[:], bdb, vn[:], start=True, stop=True)
                    prod = scr.tile([SB, NCH], F32, tag="rt", name=f"prod{it}")
                    nc.vector.tensor_mul(prod[:], bc[:], xr[:])
                    if it == 0:
                        nc.vector.reduce_sum(
                            beta[:],
                            prod[:].rearrange("p (c o) -> p c o", o=OUT),
                            axis=AX.X,
                        )
                    else:
                        binc = sp.tile([SB, CAPS], F32, tag=f"binc{it}")
                        nc.vector.reduce_sum(
                            binc[:],
                            prod[:].rearrange("p (c o) -> p c o", o=OUT),
                            axis=AX.X,
                        )
                        nc.vector.tensor_add(beta[:], beta[:], binc[:])

            # ---- transpose att -> attT [128, col=b*2+ch] --------------
            attT = sp.tile([128, 2 * B_LOC], F32, tag="attT")
            attTv = attT[:].rearrange("p (b c) -> p c b", c=2)
            for ch in range(2):
                pt2 = psB.tile([128, B_LOC], F32, tag="small")
                nc.tensor.transpose(
                    pt2[:], att_all[:, ch * 128 : (ch + 1) * 128], id4
                )
                nc.vector.tensor_copy(attTv[:, ch], pt2[:])

            attT_e = attT[:].rearrange("p (b c) -> p c b", c=2)
            nc.vector.tensor_scalar_mul(attT_e[:, 0], attT_e[:, 0], 4.0)
            nc.vector.tensor_scalar_mul(attT_e[:, 1], attT_e[:, 1], 2.0)
            # ---- scale x and store, in arrival order ------------------
            for r in range(X_TILES):
                nc.vector.tensor_scalar_mul(
                    xts[r][:], xts[r][:], attT[:, r : r + 1]
                )
                nc.scalar.dma_start(out[bass.ts(r, 128), :], xts[r][:])

    nc.compile()
    return nc


def _prep_weights(w1, b1, w2, b2):
    w1 = np.asarray(w1, dtype=np.float32)
    b1 = np.asarray(b1, dtype=np.float32)
    w2 = np.asarray(w2, dtype=np.float32)
    b2 = np.asarray(b2, dtype=np.float32)
    # w1t[i, g*512+o] = w1[g, o, i]  (the /HW mean-fold is applied when
    # pooled is copied out of PSUM, keeping w1 in fp8's normal range);
    # row CIN = b1 (contracted against the constant-one row of pooled)
    w1t = np.concatenate(
        [
            w1.transpose(2, 0, 1).reshape(CIN, G * HID),
            b1.reshape(1, G * HID),
        ],
        axis=0,
    ).astype(ml_dtypes.float8_e4m3)
    # w2t[p, g*1024 + kc*256 + mc*128] = w2[g, mc*128+o2', kc*128+p]
    w2t = np.ascontiguousarray(
        w2.transpose(0, 2, 1)
        .reshape(G, 4, 128, NCH)
        .transpose(2, 0, 1, 3)
        .reshape(128, G * 4 * NCH)
        .astype(ml_dtypes.float8_e4m3)
    )
    # b2p = [b2 grouped as (g, mc, 128)..., 1, 1, 1, 1]
    b2p = np.concatenate(
        [b2.reshape(1, 2 * G * 128), np.ones((1, B_LOC), np.float32)], axis=1
    ).astype(ml_dtypes.bfloat16)
    return np.ascontiguousarray(w1t), w2t, np.ascontiguousarray(b2p)


def make_in_maps(embedding, x, w1, b1, w2, b2):
    embedding = np.asarray(embedding, dtype=np.float32).reshape(B, CH, HW)
    # fp8 cast once, then per-sample [CH, HW] -> [HW, CH] transpose so the
    # Tensor engine can reduce over spatial (= partitions).
    emb8 = embedding.astype(ml_dtypes.float8_e4m3)
    embT = np.ascontiguousarray(emb8.transpose(0, 2, 1))     # [B, HW, CH]
    x16 = np.asarray(x, dtype=np.float32).astype(ml_dtypes.bfloat16)
    w1t, w2t, b2p = _prep_weights(w1, b1, w2, b2)
    in_maps = []
    for c in range(N_CORES):
        in_maps.append(
            {
                "emb": embT[c * B_LOC : (c + 1) * B_LOC].reshape(
                    EMBT_ROWS, 4096
                ),
                "xf16": x16[c * B_LOC : (c + 1) * B_LOC].reshape(X_ROWS // 2, HW),
                "xf8": x8[c * B_LOC : (c + 1) * B_LOC].reshape(X_ROWS // 2, HW),
                "w1t": w1t,
                "w2t": w2t,
                "b2p": b2p,
            }
        )
    return in_maps


def kernel(embedding, x, w1, b1, w2, b2):
    # This axon client has no NTFF profiling hook; a stray BASS_TRACE in the
    # environment would crash run_bass_kernel_spmd's trace path.
    os.environ.setdefault("BASS_NEVER_TRACE", "1")
    nc = build_nc()
    in_maps = make_in_maps(embedding, x, w1, b1, w2, b2)
    res = run_bass_kernel_spmd(nc, in_maps, core_ids=list(range(N_CORES)))
    out = np.concatenate(
        [
            (r["out"].astype(np.float32) * 0.25).reshape(B_LOC, NCH, 64, 64)
            for r in res.results
        ],
        axis=0,
    )
    return out
